# revision 5
# baseline (speedup 1.0000x reference)
"""GATv2 (2-layer, N=100, B=8) Trainium2 Bass kernel, 8-core SPMD.

Strategy (v4, dense-128 DoubleRow + single-ring bulk DMA + warm PE):
  * The two [10000,10000] f32 lin_n_node matrices dominate.  edge_att_L =
    tanh(inv @ WnL.T) depends only on adj_mat, so both big matmuls are
    tensor-parallel sharded over the output dim: core c owns 1250 columns
    of WnL.T in fp8e4 (x256 scale).  The contraction dim m = 10000 is
    host-packed DENSELY over all 128 partitions: m = 80*p + 2*c + d with
    p in [0,128), chunk c in [0,40), DoubleRow pair d in {0,1} -- every
    PE column-cycle carries 256 live fp8 values (vs 200 for the naive
    [100 x 100] split).  inv (x 2^-2, fp8) is repartitioned on-chip into
    the same order via a 160KB DRAM bounce, and serves as the stationary
    [128, 2, 16] per chunk.  The x2^6 net scale is undone by the tanh's
    input scale.  An AllToAll (bf16) then hands core c the full [10000]
    row for batch c.
  * Weight streaming is 10 group-DMAs of 1.29MB per layer ([128 parts x
    10112B contiguous per partition]) on the sync HWDGE ring alone --
    near line rate.  The scalar (ACT) ring carries only small latency-
    critical DMAs; gpsimd carries e-bounces and collective triggers.
  * A burst of dummy matmuls at t=0 lifts the PE out of the HAM 1.2GHz
    cold state before streaming starts, and the schedule keeps PE gaps
    well under the 3.4us re-throttle window.
  * Both layers stream back-to-back; a2a_1 flies while layer-2 streams;
    attention-1 / o1 / g2 / e2 overlap inside the layer-2 stream.
"""

import sys

for p in ("/opt/trn_rl_repo", "/opt/pypackages"):
    if p not in sys.path:
        sys.path.insert(0, p)

import numpy as np

import concourse.bass as bass
import concourse.mybir as mybir
import concourse.tile as tile
from concourse import bacc
from concourse.bass_utils import run_bass_kernel_spmd

F32 = mybir.dt.float32
BF16 = mybir.dt.bfloat16
FP8 = mybir.dt.float8e4
AF = mybir.ActivationFunctionType
ALU = mybir.AluOpType
DR = mybir.MatmulPerfMode.DoubleRow

N = 100
N2 = N * N
B = 8
NCORE = 8
SH = N2 // NCORE          # 1250 output columns per core
DH = 128                  # hidden dim
INF_ = 64                 # input features
MPAD = 10240              # m padded to 128 * 80
MP = 80                   # m-values per partition
NCHUNK = 40               # DoubleRow chunks (2 m per partition each)
CHKG = 4                  # chunks per DMA group
NG = NCHUNK // CHKG       # 10 group DMAs per layer
FPAD = 1264               # padded k-stride inside a DoubleRow half (16 | 1264)
GROUP_F32 = CHKG * 2 * FPAD // 4  # 2528 f32 per partition per group
IT_SLICES = [(0, 512), (512, 512), (1024, SH - 1024)]  # psum bank slices of 1250
WN_SCALE = 256.0          # host multiplies Wn by 2^8 before fp8 cast
INV_SCALE = 0.25          # kernel multiplies inv by 2^-2 before fp8 cast
TANH_SCALE = 1.0 / (WN_SCALE * INV_SCALE)  # 2^-6 undo on the psum accumulator
CHUNK_I = 10              # i-rows per e-chunk


def build_nc():
    nc = bacc.Bacc(None, num_devices=NCORE)

    # ---- kernel I/O ----
    wn1p = nc.dram_tensor("wn1p", [NG, 128, GROUP_F32], F32, kind="ExternalInput")
    wn2p = nc.dram_tensor("wn2p", [NG, 128, GROUP_F32], F32, kind="ExternalInput")
    adjt = nc.dram_tensor("adjt", [N, N, B], F32, kind="ExternalInput")   # adj[b,i,j] -> [i,j,b]
    adj_own = nc.dram_tensor("adj_own", [N, N], F32, kind="ExternalInput")  # adj[c]
    xt = nc.dram_tensor("xt", [INF_ + 1, N], F32, kind="ExternalInput")      # [x[c].T; ones]
    w_int = nc.dram_tensor("w_int", [INF_ + 1, DH], F32, kind="ExternalInput")
    wl1t = nc.dram_tensor("wl1t", [DH, DH], F32, kind="ExternalInput")
    wa1 = nc.dram_tensor("wa1", [DH, 1], F32, kind="ExternalInput")
    w2t = nc.dram_tensor("w2t", [2 * DH, 2 * DH], F32, kind="ExternalInput")
    b2 = nc.dram_tensor("b2", [DH, 2], F32, kind="ExternalInput")
    wl2t = nc.dram_tensor("wl2t", [2 * DH, DH], F32, kind="ExternalInput")
    wa2 = nc.dram_tensor("wa2", [DH, 1], F32, kind="ExternalInput")
    wm1t = nc.dram_tensor("wm1t", [3 * DH, 2 * DH], F32, kind="ExternalInput")
    bm1 = nc.dram_tensor("bm1", [DH, 2], F32, kind="ExternalInput")
    wm2t = nc.dram_tensor("wm2t", [2 * DH, DH], F32, kind="ExternalInput")
    bm2 = nc.dram_tensor("bm2", [DH, 1], F32, kind="ExternalInput")
    wm3t = nc.dram_tensor("wm3t", [DH, 2], F32, kind="ExternalInput")
    bm3 = nc.dram_tensor("bm3", [2, 1], F32, kind="ExternalInput")
    ident = nc.dram_tensor("ident", [128, 128], F32, kind="ExternalInput")
    eye100 = nc.dram_tensor("eye100", [N, N], F32, kind="ExternalInput")  # 1.0*I
    out_ext = nc.dram_tensor("out", [N, 2], F32, kind="ExternalOutput")

    with tile.TileContext(nc) as tc:
        with (
            tc.tile_pool(name="const", bufs=1) as cpool,
            tc.tile_pool(name="prep", bufs=1) as prep,
            tc.tile_pool(name="state", bufs=1) as state,
            tc.tile_pool(name="work", bufs=2) as work,
            tc.tile_pool(name="epool", bufs=3) as epool,
            tc.tile_pool(name="wn", bufs=10) as wnp,
            tc.tile_pool(name="psbig", bufs=1, space="PSUM") as psbig,
            tc.tile_pool(name="pssm", bufs=5, space="PSUM") as pssm,
            tc.tile_pool(name="dram", bufs=1, space="DRAM") as dram,
        ):
            # =============================================================
            # Bulk weight streams: all 20 group DMAs on the sync HWDGE ring
            # (FIFO, paced by the wn pool's 8 buffers).  Nothing else ever
            # rides this ring, so descriptor supply is continuous.
            # =============================================================
            _wn_dram = {1: wn1p, 2: wn2p}
            _wn_tiles = {1: {}, 2: {}}

            def dma_group(tag, g):
                wtile = wnp.tile([128, CHKG, 2 * FPAD // 4], F32, name="wn")
                nc.sync.dma_start(
                    wtile[:],
                    _wn_dram[tag][g].rearrange("p (a b) -> p a b", a=CHKG),
                )
                _wn_tiles[tag][g] = wtile

            for g in range(NG):
                dma_group(1, g)
            for g in range(NG):
                dma_group(2, g)

            # ---- critical constants (scalar ring; small + early) ----
            def cload(name, dt_, shape, src, eng=None):
                t = cpool.tile(shape, dt_, name=name)
                (eng or nc.scalar).dma_start(t[:], src[:])
                return t

            def cload_kt(name, src, kt, m, eng=None):
                t = cpool.tile([128, kt, m], F32, name=name)
                (eng or nc.gpsimd).dma_start(t[:], src[:].rearrange("(k p) m -> p k m", p=128))
                return t

            adj_sb = cload("adj_sb", F32, [N, N, B], adjt)
            eye_sb = cload("eye_sb", F32, [N, N], eye100)
            adjo_sb = cload("adjo_sb", F32, [N, N], adj_own)
            xt_sb = cload("xt_sb", F32, [INF_ + 1, N], xt)
            w_int_sb = cload("w_int_sb", F32, [INF_ + 1, DH], w_int)
            wl1t_sb = cload("wl1t_sb", F32, [DH, DH], wl1t)
            wa1_sb = cload("wa1_sb", F32, [DH, 1], wa1)
            id_sb = cload("id_sb", F32, [128, 128], ident)

            # ---- PE warmup: lift HAM out of the 1.2GHz cold state ----
            warm = cpool.tile([128, 512], F32, name="warm")
            nc.vector.memset(warm[:], 0.0)
            for _ in range(8):
                pw = pssm.tile([16, 512], F32, name="ps")
                nc.tensor.matmul(pw[:], warm[:, 0:16], warm[:], start=True, stop=True)

            # preload ACT LUTs (sqrt/tanh/exp) so first real use isn't blocked
            tscr = work.tile([1, 4], F32, name="tscr")
            nc.vector.memset(tscr[:], 1.0)
            nc.scalar.sqrt(tscr[:], tscr[:])
            nc.scalar.activation(tscr[:], tscr[:], AF.Tanh)
            nc.scalar.activation(tscr[:], tscr[:], AF.Exp)

            # =============================================================
            # Stage A: adj -> inv8 (adj entries are {0,1}).
            #   inv = 0.25 * norm_row * (adj + I.(2 - 7/3 adj))
            #   norm_row^2 = rowsum(adj) + diag(adj) + 0.25
            # Then repartition [100,100,16] -> [128, 80, 16] (m = 80p + q)
            # through a 160KB DRAM bounce so every partition carries live m.
            # =============================================================
            eyeb = eye_sb[:, :, None].to_broadcast([N, N, B])
            s_t = prep.tile([N, N, B], F32, name="s_t")
            nc.vector.tensor_scalar(s_t[:], adj_sb[:], -7.0 / 3.0, 2.0, ALU.mult, ALU.add)
            nc.vector.tensor_tensor(s_t[:], eyeb, s_t[:], ALU.mult)
            w_t = s_t
            nc.vector.tensor_tensor(w_t[:], adj_sb[:], s_t[:], ALU.add)

            d1 = prep.tile([N, N, B], F32, name="d1")
            nc.vector.tensor_tensor(d1[:], eyeb, adj_sb[:], ALU.mult)
            nc.vector.tensor_tensor(d1[:], adj_sb[:], d1[:], ALU.add)
            nsq = prep.tile([N, B], F32, name="nsq")
            nc.vector.tensor_reduce(nsq[:], d1[:].rearrange("i j b -> i b j"),
                                    axis=mybir.AxisListType.X, op=ALU.add)
            nc.vector.tensor_scalar(nsq[:], nsq[:], 1.0, 0.25, ALU.mult, ALU.add)
            # nwt = 0.25*sqrt(nsq) via the ACT input scale: sqrt(nsq/16)
            nwt = prep.tile([N, B], F32, name="nwt")
            nc.scalar.activation(nwt[:], nsq[:], AF.Sqrt,
                                 scale=INV_SCALE * INV_SCALE)

            inv8 = state.tile([N, N, 16], FP8, name="inv8")
            nc.vector.memset(inv8[:], 0.0)
            nc.vector.tensor_tensor(inv8[:, :, 0:B], w_t[:],
                                    nwt[:, None, :].to_broadcast([N, N, B]), ALU.mult)

            # bounce: SBUF [100,100,16] -> DRAM flat (m,b) -> SBUF [128,80,16]
            xdram = dram.tile([N2, 16], FP8)
            nc.scalar.dma_start(
                xdram[:].rearrange("(i j) b -> i j b", j=N), inv8[:]
            )
            inv8r = state.tile([128, MP, 16], FP8, name="inv8r")
            nc.vector.memset(inv8r[:], 0.0)
            nc.scalar.dma_start(
                inv8r[0:125, :, :],
                xdram[:].rearrange("(p q) b -> p q b", q=MP),
            )

            # own-batch mask + additive -10000 bias for the e-side
            mb0 = prep.tile([N, N], F32, name="mb0")
            nc.vector.tensor_tensor(mb0[:], adjo_sb[:], eye_sb[:], ALU.add)
            maskb = prep.tile([N, N], F32, name="maskb")
            nc.vector.tensor_scalar(maskb[:], mb0[:], 1.0, None, ALU.min)
            eqz = prep.tile([N, N], F32, name="eqz")
            nc.vector.tensor_scalar(eqz[:], mb0[:], 0.0, None, ALU.is_equal)
            negm = prep.tile([N, N], F32, name="negm")
            nc.vector.tensor_scalar(negm[:], eqz[:], -10000.0, None, ALU.mult)

            # =============================================================
            # Batch-side prologue: h_inT, g1T, gnm1
            # =============================================================
            ps = pssm.tile([DH, N], F32, name="ps")
            nc.tensor.matmul(ps[:], w_int_sb[:], xt_sb[:], start=True, stop=True)
            h_inT = state.tile([DH, N], F32, name="h_inT")
            nc.vector.tensor_copy(h_inT[:], ps[:])

            ps = pssm.tile([DH, N], F32, name="ps")
            nc.tensor.matmul(ps[:], wl1t_sb[:], h_inT[:], start=True, stop=True)
            g1T = state.tile([DH, N], F32, name="g1T")
            nc.vector.tensor_copy(g1T[:], ps[:])

            def g_node_major(gT, tag):
                psg = pssm.tile([N, DH], F32, name="ps")
                nc.tensor.transpose(psg[:], gT[:], id_sb[:, :])
                gnm = state.tile([N, DH], F32, name=f"gnm{tag}")
                nc.vector.tensor_copy(gnm[:], psg[:])
                return gnm

            gnm1 = g_node_major(g1T, 1)

            # ---- remaining constants (gpsimd SWDGE; sync ring stays bulk-only)
            w2t_sb = cload_kt("w2t_sb", w2t, 2, 2 * DH)      # [128, 2, 256]
            b2_sb = cload("b2_sb", F32, [DH, 2], b2, eng=nc.gpsimd)
            wl2t_sb = cload_kt("wl2t_sb", wl2t, 2, DH)       # [128, 2, 128]
            wa2_sb = cload("wa2_sb", F32, [DH, 1], wa2, eng=nc.gpsimd)
            wm1t_sb = cload_kt("wm1t_sb", wm1t, 3, 2 * DH)   # [128, 3, 256]
            bm1_sb = cload("bm1_sb", F32, [DH, 2], bm1, eng=nc.gpsimd)
            wm2t_sb = cload_kt("wm2t_sb", wm2t, 2, DH)       # [128, 2, 128]
            bm2_sb = cload("bm2_sb", F32, [DH, 1], bm2, eng=nc.gpsimd)
            wm3t_sb = cload("wm3t_sb", F32, [DH, 2], wm3t, eng=nc.gpsimd)
            bm3_sb = cload("bm3_sb", F32, [2, 1], bm3, eng=nc.gpsimd)

            # =============================================================
            # e[i,j] = Wa . tanh(g_i + g_j), bounced to DRAM for the
            # [1, i*j] -> [i, j] repartition.
            # =============================================================
            def e_chunk(gT, wa_sb, e_dram, ci, add_eng=None):
                i0 = ci * CHUNK_I
                half = CHUNK_I // 2
                tmp = epool.tile([DH, CHUNK_I, N], F32, name="etmp")
                (add_eng or nc.vector).tensor_tensor(
                    tmp[:],
                    gT[:, i0 : i0 + CHUNK_I, None].to_broadcast([DH, CHUNK_I, N]),
                    gT[:, None, :].to_broadcast([DH, CHUNK_I, N]),
                    ALU.add,
                )
                tmp2 = epool.tile([DH, CHUNK_I, N], F32, name="etmp2")
                nc.scalar.activation(tmp2[:], tmp[:], AF.Tanh)
                eb = epool.tile([1, CHUNK_I * N], F32, name="ebounce")
                for h in range(2):
                    pe = pssm.tile([1, half * N], F32, name="ps")
                    nc.tensor.matmul(
                        pe[:], wa_sb[:],
                        tmp2[:, h * half : (h + 1) * half, :].rearrange("p a b -> p (a b)"),
                        start=True, stop=True,
                    )
                    nc.vector.tensor_copy(eb[:, h * half * N : (h + 1) * half * N], pe[:])
                nc.gpsimd.dma_start(e_dram[i0 * N : (i0 + CHUNK_I) * N], eb[0:1, :])

            e1_dram = dram.tile([N2], F32)
            e2_dram = dram.tile([N2], F32)

            # =============================================================
            # TP streams: fp8 Wn chunks, dense-128 DoubleRow accumulation
            # =============================================================
            _accs = {}

            def wn_accs():
                if "s" not in _accs:
                    _accs["s"] = [
                        psbig.tile([16, 512], F32, name=f"acc_{it}")
                        for it in range(3)
                    ]
                return _accs["s"]

            def mm_group(tag, g):
                wtile = _wn_tiles[tag][g]
                accs = wn_accs()
                for q in range(CHKG):
                    c = g * CHKG + q
                    first = c == 0
                    last = c == NCHUNK - 1
                    rhs_pair = wtile[:, q, :].bitcast(FP8).rearrange(
                        "p (two f) -> p two f", two=2
                    )
                    for it, (o, w) in enumerate(IT_SLICES):
                        nc.tensor.matmul(
                            accs[it][:, :w],
                            inv8r[:, 2 * c : 2 * c + 2, :],
                            rhs_pair[:, :, o : o + w],
                            start=first,
                            stop=last,
                            perf_mode=DR,
                        )

            def a2a_start(tag):
                accs = wn_accs()
                ea = state.tile([B, SH], BF16, name=f"ea{tag}")
                for it, (o, w) in enumerate(IT_SLICES):
                    nc.scalar.activation(ea[:, o : o + w], accs[it][0:B, :w],
                                         AF.Tanh, scale=TANH_SCALE)
                cc_in = dram.tile([B, SH], BF16)
                cc_out = dram.tile([B, SH], BF16)
                nc.scalar.dma_start(cc_in[:], ea[:])
                nc.gpsimd.collective_compute(
                    "AllToAll",
                    ALU.bypass,
                    replica_groups=[list(range(NCORE))],
                    ins=[cc_in[:].opt()],
                    outs=[cc_out[:].opt()],
                )
                return cc_out

            def a2a_finish(tag, cc_out):
                ea_ij = state.tile([N, N], BF16, name=f"eaij{tag}")
                nc.scalar.dma_start(
                    ea_ij[:], cc_out[:].rearrange("b f -> (b f)").rearrange("(i j) -> i j", j=N)
                )
                return ea_ij

            # =============================================================
            # attention + aggregation (batch side)
            # =============================================================
            def attn_dve(em, ea_ij, tag):
                ef = work.tile([N, N], F32, name=f"ef{tag}")
                nc.vector.tensor_mul(out=ef[:], in0=em[:], in1=ea_ij[:])
                nc.vector.tensor_add(out=ef[:], in0=ef[:], in1=negm[:])
                # row softmax (no max-subtraction: |ef| <= ~4 or exactly -1e4)
                aw = work.tile([N, N], F32, name=f"aw{tag}")
                nc.scalar.activation(aw[:], ef[:], AF.Exp)
                ssum = work.tile([N, 1], F32, name=f"ssum{tag}")
                nc.vector.tensor_reduce(ssum[:], aw[:], axis=mybir.AxisListType.X, op=ALU.add)
                rsum = work.tile([N, 1], F32, name=f"rsum{tag}")
                nc.vector.reciprocal(rsum[:], ssum[:])
                nc.vector.tensor_scalar_mul(aw[:], aw[:], rsum[:, 0:1])
                return aw

            def attn_pe(aw, gnm, tag):
                pst = pssm.tile([N, N], F32, name="ps")
                nc.tensor.transpose(pst[:], aw[:], id_sb[:N, :N])
                awT = work.tile([N, N], F32, name=f"awT{tag}")
                nc.vector.tensor_copy(awT[:], pst[:])
                psr = pssm.tile([DH, N], F32, name="ps")
                nc.tensor.matmul(psr[:], gnm[:], awT[:], start=True, stop=True)
                return psr

            # ---------------- schedule ----------------
            # Layer-1 stream; e1 chunks lag 3 groups so their DVE/ACT chain
            # never stalls the PE's streaming matmuls.
            for g in range(NG):
                mm_group(1, g)
                if g >= 3:
                    e_chunk(g1T, wa1_sb, e1_dram, g - 3,
                            add_eng=nc.gpsimd if g % 2 else None)

            cc1_out = a2a_start(1)

            # Layer-2 stream back-to-back (a2a_1 flies underneath); the last
            # three e1 chunks ride the first three groups.
            for g in range(NG):
                mm_group(2, g)
                if g < 3:
                    e_chunk(g1T, wa1_sb, e1_dram, 7 + g,
                            add_eng=nc.gpsimd if g % 2 else None)
                if g == 3:
                    e1_ij = state.tile([N, N], F32, name="eij1")
                    nc.scalar.dma_start(e1_ij[:], e1_dram[:].rearrange("(i j) -> i j", j=N))
                    em1 = state.tile([N, N], F32, name="em1")
                    nc.vector.tensor_mul(out=em1[:], in0=e1_ij[:], in1=maskb[:])
                    ea1_ij = a2a_finish(1, cc1_out)

            # a2a_2 kicks off the moment the last accumulation lands; its
            # ACT/scalar ops are queued ahead of the attention-1 chain so a
            # late a2a_1 can never delay it.
            cc2_out = a2a_start(2)

            aw1 = attn_dve(em1, ea1_ij, 1)
            psr1 = attn_pe(aw1, gnm1, 1)
            out1T = state.tile([DH, N], F32, name="out1T")
            nc.scalar.activation(out1T[:], psr1[:], AF.Tanh)

            # o1T = tanh(W2 @ [out1; h_in] + b2), M split in 2 halves
            o1T = []
            for mh in range(2):
                pso = pssm.tile([DH, N], F32, name="ps")
                mslc = slice(mh * DH, (mh + 1) * DH)
                nc.tensor.matmul(pso[:], w2t_sb[:, 0, mslc], out1T[:], start=True, stop=False)
                nc.tensor.matmul(pso[:], w2t_sb[:, 1, mslc], h_inT[:], start=False, stop=True)
                t = state.tile([DH, N], F32, name=f"o1T_{mh}")
                nc.scalar.activation(t[:], pso[:], AF.Tanh, bias=b2_sb[:, mh : mh + 1])
                o1T.append(t)

            # g2T = Wl2 @ o1T  (K = 256)
            psg2 = pssm.tile([DH, N], F32, name="ps")
            nc.tensor.matmul(psg2[:], wl2t_sb[:, 0, :], o1T[0][:], start=True, stop=False)
            nc.tensor.matmul(psg2[:], wl2t_sb[:, 1, :], o1T[1][:], start=False, stop=True)
            g2T = state.tile([DH, N], F32, name="g2T")
            nc.vector.tensor_copy(g2T[:], psg2[:])
            gnm2 = g_node_major(g2T, 2)

            # e2 chunks fill the a2a_2 flight window
            for ci in range(NCHUNK // CHKG):
                e_chunk(g2T, wa2_sb, e2_dram, ci,
                        add_eng=nc.gpsimd if ci % 2 else None)

            e2_ij = state.tile([N, N], F32, name="eij2")
            nc.scalar.dma_start(e2_ij[:], e2_dram[:].rearrange("(i j) -> i j", j=N))
            em2 = state.tile([N, N], F32, name="em2")
            nc.vector.tensor_mul(out=em2[:], in0=e2_ij[:], in1=maskb[:])

            ea2_ij = a2a_finish(2, cc2_out)

            # tiny warm-keeper matmuls so the HAM clock gate stays open
            # across the a2a_2 landing gap
            for _ in range(6):
                pw = pssm.tile([16, 512], F32, name="ps")
                nc.tensor.matmul(pw[:, 0:64], warm[:, 0:16], warm[:, 0:64],
                                 start=True, stop=True)

            aw2 = attn_dve(em2, ea2_ij, 2)
            psr2 = attn_pe(aw2, gnm2, 2)
            out2T = state.tile([DH, N], F32, name="out2T")
            nc.scalar.activation(out2T[:], psr2[:], AF.Tanh)

            # MLP: q1 = relu(Wm1 @ [out2; o1] + bm1)  (K=384, M=256)
            o2T_parts = [out2T, o1T[0], o1T[1]]
            q1T = []
            for mh in range(2):
                psq = pssm.tile([DH, N], F32, name="ps")
                mslc = slice(mh * DH, (mh + 1) * DH)
                for kt in range(3):
                    nc.tensor.matmul(
                        psq[:], wm1t_sb[:, kt, mslc], o2T_parts[kt][:],
                        start=(kt == 0), stop=(kt == 2),
                    )
                t = state.tile([DH, N], F32, name=f"q1T_{mh}")
                nc.scalar.activation(t[:], psq[:], AF.Relu, bias=bm1_sb[:, mh : mh + 1])
                q1T.append(t)

            # q2 = relu(Wm2 @ q1 + bm2)  (K=256, M=128)
            psq2 = pssm.tile([DH, N], F32, name="ps")
            nc.tensor.matmul(psq2[:], wm2t_sb[:, 0, :], q1T[0][:], start=True, stop=False)
            nc.tensor.matmul(psq2[:], wm2t_sb[:, 1, :], q1T[1][:], start=False, stop=True)
            q2T = state.tile([DH, N], F32, name="q2T")
            nc.scalar.activation(q2T[:], psq2[:], AF.Relu, bias=bm2_sb[:, 0:1])

            # q3 = Wm3 @ q2 + bm3  [2, 100]
            psq3 = pssm.tile([2, N], F32, name="ps")
            nc.tensor.matmul(psq3[:], wm3t_sb[:], q2T[:], start=True, stop=True)
            q3T = state.tile([2, N], F32, name="q3T")
            nc.scalar.activation(q3T[:], psq3[:], AF.Identity, bias=bm3_sb[:, 0:1])

            # transpose -> [100, 2], softmax over classes (free dim)
            psf = pssm.tile([N, 2], F32, name="ps")
            nc.tensor.transpose(psf[:], q3T[:], id_sb[:2, :2])
            qf = work.tile([N, 2], F32, name="qf")
            nc.vector.tensor_copy(qf[:], psf[:])
            fm = work.tile([N, 1], F32, name="fm")
            nc.vector.tensor_reduce(fm[:], qf[:], axis=mybir.AxisListType.X,
                                    op=ALU.max, negate=True)
            pf = work.tile([N, 2], F32, name="pf")
            nc.scalar.activation(pf[:], qf[:], AF.Exp, bias=fm[:, 0:1])
            sf = work.tile([N, 1], F32, name="sf")
            nc.vector.tensor_reduce(sf[:], pf[:], axis=mybir.AxisListType.X, op=ALU.add)
            rf = work.tile([N, 1], F32, name="rf")
            nc.vector.reciprocal(rf[:], sf[:])
            outp = work.tile([N, 2], F32, name="outp")
            nc.vector.tensor_scalar_mul(outp[:], pf[:], rf[:, 0:1])
            nc.scalar.dma_start(out_ext[:], outp[:])

    nc.compile()
    return nc


_NC_CACHE = None


def _get_nc():
    global _NC_CACHE
    if _NC_CACHE is None:
        _NC_CACHE = build_nc()
    return _NC_CACHE


def _pack_wn(WnT8, c):
    """[N2, N2] fp8 WnT -> core c's [NG, 128, GROUP_F32] dense-128 pack.

    m = 80*p + 2*(g*CHKG + q) + d; tile byte layout per partition is
    [q(4), d(2), k(1264 fp8, 1250 valid)].
    """
    sl = WnT8[:, c * SH : (c + 1) * SH]  # [10000, 1250]
    arr = np.zeros((MPAD, SH), dtype=sl.dtype)
    arr[:N2] = sl
    v = arr.reshape(128, NCHUNK, 2, SH)            # p, c, d, k
    v = v.reshape(128, NG, CHKG, 2, SH)            # p, g, q, d, k
    out = np.zeros((NG, 128, CHKG, 2, FPAD), dtype=sl.dtype)
    out[:, :, :, :, :SH] = v.transpose(1, 0, 2, 3, 4)
    return np.ascontiguousarray(out).reshape(NG, 128, CHKG * 2 * FPAD).view(np.float32)


def kernel(x, adj_mat, W_in, b_in, Wl1, Wa1, Wn1, W2, b2, Wl2, Wa2, Wn2,
           Wm1, bm1, Wm2, bm2, Wm3, bm3, _trace=False, _trace_kwargs=None):
    import ml_dtypes

    x = np.asarray(x, dtype=np.float32)
    adj_mat = np.asarray(adj_mat, dtype=np.float32)

    wn1T8 = (np.asarray(Wn1, np.float32).T * WN_SCALE).astype(ml_dtypes.float8_e4m3)
    wn2T8 = (np.asarray(Wn2, np.float32).T * WN_SCALE).astype(ml_dtypes.float8_e4m3)

    adjt = np.ascontiguousarray(adj_mat.transpose(1, 2, 0))  # [i, j, b]
    common = {
        "adjt": adjt,
        "w_int": np.ascontiguousarray(np.vstack([np.asarray(W_in, np.float32).T,
                                                 np.asarray(b_in, np.float32).reshape(1, DH)])),
        "wl1t": np.ascontiguousarray(np.asarray(Wl1, np.float32).T),
        "wa1": np.asarray(Wa1, np.float32).reshape(1, DH).T.copy(),
        "w2t": np.ascontiguousarray(np.asarray(W2, np.float32).T),
        "b2": np.ascontiguousarray(np.asarray(b2, np.float32).reshape(2, DH).T),
        "wl2t": np.ascontiguousarray(np.asarray(Wl2, np.float32).T),
        "wa2": np.asarray(Wa2, np.float32).reshape(1, DH).T.copy(),
        "wm1t": np.ascontiguousarray(np.asarray(Wm1, np.float32).T),
        "bm1": np.ascontiguousarray(np.asarray(bm1, np.float32).reshape(2, DH).T),
        "wm2t": np.ascontiguousarray(np.asarray(Wm2, np.float32).T),
        "bm2": np.asarray(bm2, np.float32).reshape(DH, 1),
        "wm3t": np.ascontiguousarray(np.asarray(Wm3, np.float32).T),
        "bm3": np.asarray(bm3, np.float32).reshape(2, 1),
        "ident": np.eye(128, dtype=np.float32),
        "eye100": np.eye(N, dtype=np.float32),
    }
    in_maps = []
    for c in range(NCORE):
        m = dict(common)
        m["wn1p"] = _pack_wn(wn1T8, c)
        m["wn2p"] = _pack_wn(wn2T8, c)
        m["adj_own"] = np.ascontiguousarray(adj_mat[c])
        m["xt"] = np.ascontiguousarray(np.vstack([x[c].T, np.ones((1, N), np.float32)]))
        in_maps.append(m)

    nc = _get_nc()
    kw = {}
    if _trace:
        kw["trace"] = True
        if _trace_kwargs:
            kw.update(_trace_kwargs)
    res = run_bass_kernel_spmd(nc, in_maps, core_ids=list(range(NCORE)), **kw)
    out = np.stack([res.results[c]["out"] for c in range(NCORE)], axis=0)
    if _trace:
        kernel._last_results = res
    return out


# revision 9
# speedup vs baseline: 1.0471x; 1.0471x over previous
"""GATv2 (2-layer, N=100, B=8) Trainium2 Bass kernel, 8-core SPMD.

Strategy (v4, dense-128 DoubleRow + single-ring bulk DMA + warm PE):
  * The two [10000,10000] f32 lin_n_node matrices dominate.  edge_att_L =
    tanh(inv @ WnL.T) depends only on adj_mat, so both big matmuls are
    tensor-parallel sharded over the output dim: core c owns 1250 columns
    of WnL.T in fp8e4 (x256 scale).  The contraction dim m = 10000 is
    host-packed DENSELY over all 128 partitions: m = 80*p + 2*c + d with
    p in [0,128), chunk c in [0,40), DoubleRow pair d in {0,1} -- every
    PE column-cycle carries 256 live fp8 values (vs 200 for the naive
    [100 x 100] split).  inv (x 2^-2, fp8) is repartitioned on-chip into
    the same order via a 160KB DRAM bounce, and serves as the stationary
    [128, 2, 16] per chunk.  The x2^6 net scale is undone by the tanh's
    input scale.  An AllToAll (bf16) then hands core c the full [10000]
    row for batch c.
  * Weight streaming is 10 group-DMAs of 1.29MB per layer ([128 parts x
    10112B contiguous per partition]) on the sync HWDGE ring alone --
    near line rate.  The scalar (ACT) ring carries only small latency-
    critical DMAs; gpsimd carries e-bounces and collective triggers.
  * A burst of dummy matmuls at t=0 lifts the PE out of the HAM 1.2GHz
    cold state before streaming starts, and the schedule keeps PE gaps
    well under the 3.4us re-throttle window.
  * Both layers stream back-to-back; a2a_1 flies while layer-2 streams;
    attention-1 / o1 / g2 / e2 overlap inside the layer-2 stream.
"""

import sys

for p in ("/opt/trn_rl_repo", "/opt/pypackages"):
    if p not in sys.path:
        sys.path.insert(0, p)

import numpy as np

import concourse.bass as bass
import concourse.mybir as mybir
import concourse.tile as tile
from concourse import bacc
from concourse.bass_utils import run_bass_kernel_spmd

F32 = mybir.dt.float32
BF16 = mybir.dt.bfloat16
FP8 = mybir.dt.float8e4
AF = mybir.ActivationFunctionType
ALU = mybir.AluOpType
DR = mybir.MatmulPerfMode.DoubleRow

N = 100
N2 = N * N
B = 8
NCORE = 8
SH = N2 // NCORE          # 1250 output columns per core
DH = 128                  # hidden dim
INF_ = 64                 # input features
MPAD = 10240              # m padded to 128 * 80
MP = 80                   # m-values per partition
NCHUNK = 40               # DoubleRow chunks (2 m per partition each)
CHKG = 4                  # chunks per DMA group
NG = NCHUNK // CHKG       # 10 group DMAs per layer
FPAD = 1264               # padded k-stride inside a DoubleRow half (16 | 1264)
GROUP_F32 = CHKG * 2 * FPAD // 4  # 2528 f32 per partition per group
IT_SLICES = [(0, 512), (512, 512), (1024, SH - 1024)]  # psum bank slices of 1250
WN_SCALE = 256.0          # host multiplies Wn by 2^8 before fp8 cast
INV_SCALE = 0.25          # kernel multiplies inv by 2^-2 before fp8 cast
TANH_SCALE = 1.0 / (WN_SCALE * INV_SCALE)  # 2^-6 undo on the psum accumulator
CHUNK_I = 10              # i-rows per e-chunk


def build_nc():
    nc = bacc.Bacc(None, num_devices=NCORE)

    # ---- kernel I/O ----
    wn1p = nc.dram_tensor("wn1p", [NG, 128, GROUP_F32], F32, kind="ExternalInput")
    wn2p = nc.dram_tensor("wn2p", [NG, 128, GROUP_F32], F32, kind="ExternalInput")
    adjt = nc.dram_tensor("adjt", [N, N, B], F32, kind="ExternalInput")   # adj[b,i,j] -> [i,j,b]
    adj_own = nc.dram_tensor("adj_own", [N, N], F32, kind="ExternalInput")  # adj[c]
    xt = nc.dram_tensor("xt", [INF_ + 1, N], F32, kind="ExternalInput")      # [x[c].T; ones]
    w_int = nc.dram_tensor("w_int", [INF_ + 1, DH], F32, kind="ExternalInput")
    wl1t = nc.dram_tensor("wl1t", [DH, DH], F32, kind="ExternalInput")
    wa1 = nc.dram_tensor("wa1", [DH, 1], F32, kind="ExternalInput")
    w2t = nc.dram_tensor("w2t", [2 * DH, 2 * DH], F32, kind="ExternalInput")
    b2 = nc.dram_tensor("b2", [DH, 2], F32, kind="ExternalInput")
    wl2t = nc.dram_tensor("wl2t", [2 * DH, DH], F32, kind="ExternalInput")
    wa2 = nc.dram_tensor("wa2", [DH, 1], F32, kind="ExternalInput")
    wm1t = nc.dram_tensor("wm1t", [3 * DH, 2 * DH], F32, kind="ExternalInput")
    bm1 = nc.dram_tensor("bm1", [DH, 2], F32, kind="ExternalInput")
    wm2t = nc.dram_tensor("wm2t", [2 * DH, DH], F32, kind="ExternalInput")
    bm2 = nc.dram_tensor("bm2", [DH, 1], F32, kind="ExternalInput")
    wm3t = nc.dram_tensor("wm3t", [DH, 2], F32, kind="ExternalInput")
    bm3 = nc.dram_tensor("bm3", [2, 1], F32, kind="ExternalInput")
    ident = nc.dram_tensor("ident", [128, 128], F32, kind="ExternalInput")
    eye100 = nc.dram_tensor("eye100", [N, N], F32, kind="ExternalInput")  # 1.0*I
    out_ext = nc.dram_tensor("out", [N, 2], F32, kind="ExternalOutput")

    with tile.TileContext(nc) as tc:
        with (
            tc.tile_pool(name="const", bufs=1) as cpool,
            tc.tile_pool(name="prep", bufs=1) as prep,
            tc.tile_pool(name="state", bufs=1) as state,
            tc.tile_pool(name="work", bufs=2) as work,
            tc.tile_pool(name="epool", bufs=3) as epool,
            tc.tile_pool(name="wn", bufs=10) as wnp,
            tc.tile_pool(name="psbig", bufs=1, space="PSUM") as psbig,
            tc.tile_pool(name="pssm", bufs=5, space="PSUM") as pssm,
            tc.tile_pool(name="dram", bufs=1, space="DRAM") as dram,
        ):
            # =============================================================
            # Bulk weight streams: all 20 group DMAs on the sync HWDGE ring
            # (FIFO, paced by the wn pool's 8 buffers).  Nothing else ever
            # rides this ring, so descriptor supply is continuous.
            # =============================================================
            _wn_dram = {1: wn1p, 2: wn2p}
            _wn_tiles = {1: {}, 2: {}}

            def dma_group(tag, g):
                wtile = wnp.tile([128, CHKG, 2 * FPAD // 4], F32, name="wn")
                nc.sync.dma_start(
                    wtile[:],
                    _wn_dram[tag][g].rearrange("p (a b) -> p a b", a=CHKG),
                )
                _wn_tiles[tag][g] = wtile

            for g in range(NG):
                dma_group(1, g)
            for g in range(NG):
                dma_group(2, g)

            # ---- critical constants (scalar ring; small + early) ----
            def cload(name, dt_, shape, src, eng=None):
                t = cpool.tile(shape, dt_, name=name)
                (eng or nc.scalar).dma_start(t[:], src[:])
                return t

            def cload_kt(name, src, kt, m, eng=None):
                t = cpool.tile([128, kt, m], F32, name=name)
                (eng or nc.gpsimd).dma_start(t[:], src[:].rearrange("(k p) m -> p k m", p=128))
                return t

            adj_sb = cload("adj_sb", F32, [N, N, B], adjt)
            eye_sb = cload("eye_sb", F32, [N, N], eye100)
            adjo_sb = cload("adjo_sb", F32, [N, N], adj_own)
            xt_sb = cload("xt_sb", F32, [INF_ + 1, N], xt)
            w_int_sb = cload("w_int_sb", F32, [INF_ + 1, DH], w_int)
            wl1t_sb = cload("wl1t_sb", F32, [DH, DH], wl1t)
            wa1_sb = cload("wa1_sb", F32, [DH, 1], wa1)
            id_sb = cload("id_sb", F32, [128, 128], ident)

            # ---- PE warmup: lift HAM out of the 1.2GHz cold state ----
            warm = cpool.tile([128, 512], F32, name="warm")
            nc.vector.memset(warm[:], 0.0)
            for _ in range(8):
                pw = pssm.tile([16, 512], F32, name="ps")
                nc.tensor.matmul(pw[:], warm[:, 0:16], warm[:], start=True, stop=True)

            # preload ACT LUTs (sqrt/tanh/exp) so first real use isn't blocked
            tscr = work.tile([1, 4], F32, name="tscr")
            nc.vector.memset(tscr[:], 1.0)
            nc.scalar.sqrt(tscr[:], tscr[:])
            nc.scalar.activation(tscr[:], tscr[:], AF.Tanh)
            nc.scalar.activation(tscr[:], tscr[:], AF.Exp)

            # =============================================================
            # Stage A: adj -> inv8 (adj entries are {0,1}).
            #   inv = 0.25 * norm_row * (adj + I.(2 - 7/3 adj))
            #   norm_row^2 = rowsum(adj) + diag(adj) + 0.25
            # Then repartition [100,100,16] -> [128, 80, 16] (m = 80p + q)
            # through a 160KB DRAM bounce so every partition carries live m.
            # =============================================================
            eyeb = eye_sb[:, :, None].to_broadcast([N, N, B])
            s_t = prep.tile([N, N, B], F32, name="s_t")
            nc.vector.tensor_scalar(s_t[:], adj_sb[:], -7.0 / 3.0, 2.0, ALU.mult, ALU.add)
            nc.vector.tensor_tensor(s_t[:], eyeb, s_t[:], ALU.mult)
            w_t = s_t
            nc.vector.tensor_tensor(w_t[:], adj_sb[:], s_t[:], ALU.add)

            d1 = prep.tile([N, N, B], F32, name="d1")
            nc.vector.tensor_tensor(d1[:], eyeb, adj_sb[:], ALU.mult)
            nc.vector.tensor_tensor(d1[:], adj_sb[:], d1[:], ALU.add)
            nsq = prep.tile([N, B], F32, name="nsq")
            nc.vector.tensor_reduce(nsq[:], d1[:].rearrange("i j b -> i b j"),
                                    axis=mybir.AxisListType.X, op=ALU.add)
            nc.vector.tensor_scalar(nsq[:], nsq[:], 1.0, 0.25, ALU.mult, ALU.add)
            # nwt = 0.25*sqrt(nsq) via the ACT input scale: sqrt(nsq/16)
            nwt = prep.tile([N, B], F32, name="nwt")
            nc.scalar.activation(nwt[:], nsq[:], AF.Sqrt,
                                 scale=INV_SCALE * INV_SCALE)

            inv8 = state.tile([N, N, 16], FP8, name="inv8")
            nc.vector.memset(inv8[:], 0.0)
            nc.vector.tensor_tensor(inv8[:, :, 0:B], w_t[:],
                                    nwt[:, None, :].to_broadcast([N, N, B]), ALU.mult)

            # bounce: SBUF [100,100,16] -> DRAM flat (m,b) -> SBUF [128,80,16]
            xdram = dram.tile([N2, 16], FP8)
            nc.scalar.dma_start(
                xdram[:].rearrange("(i j) b -> i j b", j=N), inv8[:]
            )
            inv8r = state.tile([128, MP, 16], FP8, name="inv8r")
            nc.vector.memset(inv8r[:], 0.0)
            nc.scalar.dma_start(
                inv8r[0:125, :, :],
                xdram[:].rearrange("(p q) b -> p q b", q=MP),
            )

            # own-batch mask + additive -10000 bias for the e-side
            mb0 = prep.tile([N, N], F32, name="mb0")
            nc.vector.tensor_tensor(mb0[:], adjo_sb[:], eye_sb[:], ALU.add)
            maskb = prep.tile([N, N], F32, name="maskb")
            nc.vector.tensor_scalar(maskb[:], mb0[:], 1.0, None, ALU.min)
            eqz = prep.tile([N, N], F32, name="eqz")
            nc.vector.tensor_scalar(eqz[:], mb0[:], 0.0, None, ALU.is_equal)
            negm = prep.tile([N, N], F32, name="negm")
            nc.vector.tensor_scalar(negm[:], eqz[:], -10000.0, None, ALU.mult)

            # =============================================================
            # Batch-side prologue: h_inT, g1T, gnm1
            # =============================================================
            ps = pssm.tile([DH, N], F32, name="ps")
            nc.tensor.matmul(ps[:], w_int_sb[:], xt_sb[:], start=True, stop=True)
            h_inT = state.tile([DH, N], F32, name="h_inT")
            nc.vector.tensor_copy(h_inT[:], ps[:])

            ps = pssm.tile([DH, N], F32, name="ps")
            nc.tensor.matmul(ps[:], wl1t_sb[:], h_inT[:], start=True, stop=True)
            g1T = state.tile([DH, N], F32, name="g1T")
            nc.vector.tensor_copy(g1T[:], ps[:])

            def g_node_major(gT, tag):
                psg = pssm.tile([N, DH], F32, name="ps")
                nc.tensor.transpose(psg[:], gT[:], id_sb[:, :])
                gnm = state.tile([N, DH], F32, name=f"gnm{tag}")
                nc.vector.tensor_copy(gnm[:], psg[:])
                return gnm

            gnm1 = g_node_major(g1T, 1)
            g1Tb = state.tile([DH, N], BF16, name="g1Tb")
            nc.vector.tensor_copy(g1Tb[:], g1T[:])
            wa1b = cpool.tile([DH, 1], BF16, name="wa1b")
            nc.vector.tensor_copy(wa1b[:], wa1_sb[:])

            # =============================================================
            # e[i,j] = Wa . tanh(g_i + g_j), bounced to DRAM for the
            # [1, i*j] -> [i, j] repartition.  The add/tanh pipe runs in
            # bf16 (2x DVE rate); adds alternate gpsimd/DVE and the psum
            # evacuations alternate DVE/ACT so no single engine gates the
            # PE's interleaved e-matmuls.
            # =============================================================
            def e_chunk(gTb, wa_b, e_dram, ci):
                i0 = ci * CHUNK_I
                half = CHUNK_I // 2
                tmp = epool.tile([DH, CHUNK_I, N], BF16, name="etmp")
                (nc.gpsimd if ci % 2 == 0 else nc.vector).tensor_tensor(
                    tmp[:],
                    gTb[:, i0 : i0 + CHUNK_I, None].to_broadcast([DH, CHUNK_I, N]),
                    gTb[:, None, :].to_broadcast([DH, CHUNK_I, N]),
                    ALU.add,
                )
                tmp2 = epool.tile([DH, CHUNK_I, N], BF16, name="etmp2")
                nc.scalar.activation(tmp2[:], tmp[:], AF.Tanh)
                eb = epool.tile([1, CHUNK_I * N], F32, name="ebounce")
                for h in range(2):
                    pe = pssm.tile([1, half * N], F32, name="ps")
                    nc.tensor.matmul(
                        pe[:], wa_b[:],
                        tmp2[:, h * half : (h + 1) * half, :].rearrange("p a b -> p (a b)"),
                        start=True, stop=True,
                    )
                    dst = eb[:, h * half * N : (h + 1) * half * N]
                    if h == 0:
                        nc.vector.tensor_copy(dst, pe[:])
                    else:
                        nc.scalar.activation(dst, pe[:], AF.Identity)
                nc.gpsimd.dma_start(e_dram[i0 * N : (i0 + CHUNK_I) * N], eb[0:1, :])

            e1_dram = dram.tile([N2], F32)
            e2_dram = dram.tile([N2], F32)

            # =============================================================
            # TP streams: fp8 Wn chunks, dense-128 DoubleRow accumulation
            # =============================================================
            _accs = {}

            def wn_accs():
                if "s" not in _accs:
                    _accs["s"] = [
                        psbig.tile([16, 512], F32, name=f"acc_{it}")
                        for it in range(3)
                    ]
                return _accs["s"]

            def mm_group(tag, g):
                wtile = _wn_tiles[tag][g]
                accs = wn_accs()
                for q in range(CHKG):
                    c = g * CHKG + q
                    first = c == 0
                    last = c == NCHUNK - 1
                    rhs_pair = wtile[:, q, :].bitcast(FP8).rearrange(
                        "p (two f) -> p two f", two=2
                    )
                    for it, (o, w) in enumerate(IT_SLICES):
                        nc.tensor.matmul(
                            accs[it][:, :w],
                            inv8r[:, 2 * c : 2 * c + 2, :],
                            rhs_pair[:, :, o : o + w],
                            start=first,
                            stop=last,
                            perf_mode=DR,
                        )

            def a2a_start(tag):
                accs = wn_accs()
                ea = state.tile([B, SH], BF16, name=f"ea{tag}")
                for it, (o, w) in enumerate(IT_SLICES):
                    nc.scalar.activation(ea[:, o : o + w], accs[it][0:B, :w],
                                         AF.Tanh, scale=TANH_SCALE)
                cc_in = dram.tile([B, SH], BF16)
                cc_out = dram.tile([B, SH], BF16)
                nc.scalar.dma_start(cc_in[:], ea[:])
                nc.gpsimd.collective_compute(
                    "AllToAll",
                    ALU.bypass,
                    replica_groups=[list(range(NCORE))],
                    ins=[cc_in[:].opt()],
                    outs=[cc_out[:].opt()],
                )
                return cc_out

            def a2a_finish(tag, cc_out):
                ea_ij = state.tile([N, N], BF16, name=f"eaij{tag}")
                nc.scalar.dma_start(
                    ea_ij[:], cc_out[:].rearrange("b f -> (b f)").rearrange("(i j) -> i j", j=N)
                )
                return ea_ij

            # =============================================================
            # attention + aggregation (batch side)
            # =============================================================
            def attn_dve(em, ea_ij, tag):
                ef = work.tile([N, N], F32, name=f"ef{tag}")
                nc.vector.tensor_mul(out=ef[:], in0=em[:], in1=ea_ij[:])
                nc.vector.tensor_add(out=ef[:], in0=ef[:], in1=negm[:])
                # row softmax (no max-subtraction: |ef| <= ~4 or exactly -1e4)
                aw = work.tile([N, N], F32, name=f"aw{tag}")
                nc.scalar.activation(aw[:], ef[:], AF.Exp)
                ssum = work.tile([N, 1], F32, name=f"ssum{tag}")
                nc.vector.tensor_reduce(ssum[:], aw[:], axis=mybir.AxisListType.X, op=ALU.add)
                rsum = work.tile([N, 1], F32, name=f"rsum{tag}")
                nc.vector.reciprocal(rsum[:], ssum[:])
                nc.vector.tensor_scalar_mul(aw[:], aw[:], rsum[:, 0:1])
                return aw

            def attn_pe(aw, gnm, tag):
                pst = pssm.tile([N, N], F32, name="ps")
                nc.tensor.transpose(pst[:], aw[:], id_sb[:N, :N])
                awT = work.tile([N, N], F32, name=f"awT{tag}")
                nc.vector.tensor_copy(awT[:], pst[:])
                psr = pssm.tile([DH, N], F32, name="ps")
                nc.tensor.matmul(psr[:], gnm[:], awT[:], start=True, stop=True)
                return psr

            # ---------------- schedule ----------------
            # Layer-1 stream with e1 chunks interleaved (their chain runs on
            # gpsimd/DVE/ACT which are otherwise idle during the DMA-paced
            # stream).
            for g in range(NG):
                mm_group(1, g)
                e_chunk(g1Tb, wa1b, e1_dram, g)

            cc1_out = a2a_start(1)

            # deferred constants (gpsimd is clear of e1 adds by now)
            w2t_sb = cload_kt("w2t_sb", w2t, 2, 2 * DH)      # [128, 2, 256]
            b2_sb = cload("b2_sb", F32, [DH, 2], b2, eng=nc.gpsimd)
            wl2t_sb = cload_kt("wl2t_sb", wl2t, 2, DH)       # [128, 2, 128]
            wa2_sb = cload("wa2_sb", F32, [DH, 1], wa2, eng=nc.gpsimd)
            wm1t_sb = cload_kt("wm1t_sb", wm1t, 3, 2 * DH)   # [128, 3, 256]
            bm1_sb = cload("bm1_sb", F32, [DH, 2], bm1, eng=nc.gpsimd)
            wm2t_sb = cload_kt("wm2t_sb", wm2t, 2, DH)       # [128, 2, 128]
            bm2_sb = cload("bm2_sb", F32, [DH, 1], bm2, eng=nc.gpsimd)
            wm3t_sb = cload("wm3t_sb", F32, [DH, 2], wm3t, eng=nc.gpsimd)
            bm3_sb = cload("bm3_sb", F32, [2, 1], bm3, eng=nc.gpsimd)

            # warm-keepers across the L1->L2 DMA seam (the ring needs ~4us
            # to deliver the first layer-2 group after the last layer-1 one)
            for _ in range(8):
                pw = pssm.tile([16, 512], F32, name="ps")
                nc.tensor.matmul(pw[:, 0:64], warm[:, 0:16], warm[:, 0:64],
                                 start=True, stop=True)

            # Layer-2 stream back-to-back (a2a_1 flies underneath)
            for g in range(NG):
                mm_group(2, g)
                if g == 0:
                    e1_ij = state.tile([N, N], F32, name="eij1")
                    nc.scalar.dma_start(e1_ij[:], e1_dram[:].rearrange("(i j) -> i j", j=N))
                    em1 = state.tile([N, N], F32, name="em1")
                    nc.vector.tensor_mul(out=em1[:], in0=e1_ij[:], in1=maskb[:])
                    ea1_ij = a2a_finish(1, cc1_out)

            # a2a_2 kicks off the moment the last accumulation lands; its
            # ACT/scalar ops are queued ahead of the attention-1 chain so a
            # late a2a_1 can never delay it.
            cc2_out = a2a_start(2)

            aw1 = attn_dve(em1, ea1_ij, 1)
            psr1 = attn_pe(aw1, gnm1, 1)
            out1T = state.tile([DH, N], F32, name="out1T")
            nc.scalar.activation(out1T[:], psr1[:], AF.Tanh)

            # o1T = tanh(W2 @ [out1; h_in] + b2), M split in 2 halves
            o1T = []
            for mh in range(2):
                pso = pssm.tile([DH, N], F32, name="ps")
                mslc = slice(mh * DH, (mh + 1) * DH)
                nc.tensor.matmul(pso[:], w2t_sb[:, 0, mslc], out1T[:], start=True, stop=False)
                nc.tensor.matmul(pso[:], w2t_sb[:, 1, mslc], h_inT[:], start=False, stop=True)
                t = state.tile([DH, N], F32, name=f"o1T_{mh}")
                nc.scalar.activation(t[:], pso[:], AF.Tanh, bias=b2_sb[:, mh : mh + 1])
                o1T.append(t)

            # g2T = Wl2 @ o1T  (K = 256)
            psg2 = pssm.tile([DH, N], F32, name="ps")
            nc.tensor.matmul(psg2[:], wl2t_sb[:, 0, :], o1T[0][:], start=True, stop=False)
            nc.tensor.matmul(psg2[:], wl2t_sb[:, 1, :], o1T[1][:], start=False, stop=True)
            g2T = state.tile([DH, N], F32, name="g2T")
            nc.vector.tensor_copy(g2T[:], psg2[:])
            gnm2 = g_node_major(g2T, 2)
            g2Tb = state.tile([DH, N], BF16, name="g2Tb")
            nc.vector.tensor_copy(g2Tb[:], psg2[:])
            wa2b = cpool.tile([DH, 1], BF16, name="wa2b")
            nc.vector.tensor_copy(wa2b[:], wa2_sb[:])

            # e2 chunks fill the a2a_2 flight window
            for ci in range(N // CHUNK_I):
                e_chunk(g2Tb, wa2b, e2_dram, ci)

            e2_ij = state.tile([N, N], F32, name="eij2")
            nc.scalar.dma_start(e2_ij[:], e2_dram[:].rearrange("(i j) -> i j", j=N))
            em2 = state.tile([N, N], F32, name="em2")
            nc.vector.tensor_mul(out=em2[:], in0=e2_ij[:], in1=maskb[:])

            ea2_ij = a2a_finish(2, cc2_out)

            # tiny warm-keeper matmuls so the HAM clock gate stays open
            # across the a2a_2 landing gap
            for _ in range(6):
                pw = pssm.tile([16, 512], F32, name="ps")
                nc.tensor.matmul(pw[:, 0:64], warm[:, 0:16], warm[:, 0:64],
                                 start=True, stop=True)

            aw2 = attn_dve(em2, ea2_ij, 2)
            psr2 = attn_pe(aw2, gnm2, 2)
            out2T = state.tile([DH, N], F32, name="out2T")
            nc.scalar.activation(out2T[:], psr2[:], AF.Tanh)

            # MLP: q1 = relu(Wm1 @ [out2; o1] + bm1)  (K=384, M=256)
            o2T_parts = [out2T, o1T[0], o1T[1]]
            q1T = []
            for mh in range(2):
                psq = pssm.tile([DH, N], F32, name="ps")
                mslc = slice(mh * DH, (mh + 1) * DH)
                for kt in range(3):
                    nc.tensor.matmul(
                        psq[:], wm1t_sb[:, kt, mslc], o2T_parts[kt][:],
                        start=(kt == 0), stop=(kt == 2),
                    )
                t = state.tile([DH, N], F32, name=f"q1T_{mh}")
                nc.scalar.activation(t[:], psq[:], AF.Relu, bias=bm1_sb[:, mh : mh + 1])
                q1T.append(t)

            # q2 = relu(Wm2 @ q1 + bm2)  (K=256, M=128)
            psq2 = pssm.tile([DH, N], F32, name="ps")
            nc.tensor.matmul(psq2[:], wm2t_sb[:, 0, :], q1T[0][:], start=True, stop=False)
            nc.tensor.matmul(psq2[:], wm2t_sb[:, 1, :], q1T[1][:], start=False, stop=True)
            q2T = state.tile([DH, N], F32, name="q2T")
            nc.scalar.activation(q2T[:], psq2[:], AF.Relu, bias=bm2_sb[:, 0:1])

            # q3 = Wm3 @ q2 + bm3  [2, 100]
            psq3 = pssm.tile([2, N], F32, name="ps")
            nc.tensor.matmul(psq3[:], wm3t_sb[:], q2T[:], start=True, stop=True)
            q3T = state.tile([2, N], F32, name="q3T")
            nc.scalar.activation(q3T[:], psq3[:], AF.Identity, bias=bm3_sb[:, 0:1])

            # transpose -> [100, 2], softmax over classes (free dim)
            psf = pssm.tile([N, 2], F32, name="ps")
            nc.tensor.transpose(psf[:], q3T[:], id_sb[:2, :2])
            qf = work.tile([N, 2], F32, name="qf")
            nc.vector.tensor_copy(qf[:], psf[:])
            fm = work.tile([N, 1], F32, name="fm")
            nc.vector.tensor_reduce(fm[:], qf[:], axis=mybir.AxisListType.X,
                                    op=ALU.max, negate=True)
            pf = work.tile([N, 2], F32, name="pf")
            nc.scalar.activation(pf[:], qf[:], AF.Exp, bias=fm[:, 0:1])
            sf = work.tile([N, 1], F32, name="sf")
            nc.vector.tensor_reduce(sf[:], pf[:], axis=mybir.AxisListType.X, op=ALU.add)
            rf = work.tile([N, 1], F32, name="rf")
            nc.vector.reciprocal(rf[:], sf[:])
            outp = work.tile([N, 2], F32, name="outp")
            nc.vector.tensor_scalar_mul(outp[:], pf[:], rf[:, 0:1])
            nc.scalar.dma_start(out_ext[:], outp[:])

    nc.compile()
    return nc


_NC_CACHE = None


def _get_nc():
    global _NC_CACHE
    if _NC_CACHE is None:
        _NC_CACHE = build_nc()
    return _NC_CACHE


def _pack_wn(WnT8, c):
    """[N2, N2] fp8 WnT -> core c's [NG, 128, GROUP_F32] dense-128 pack.

    m = 80*p + 2*(g*CHKG + q) + d; tile byte layout per partition is
    [q(4), d(2), k(1264 fp8, 1250 valid)].
    """
    sl = WnT8[:, c * SH : (c + 1) * SH]  # [10000, 1250]
    arr = np.zeros((MPAD, SH), dtype=sl.dtype)
    arr[:N2] = sl
    v = arr.reshape(128, NCHUNK, 2, SH)            # p, c, d, k
    v = v.reshape(128, NG, CHKG, 2, SH)            # p, g, q, d, k
    out = np.zeros((NG, 128, CHKG, 2, FPAD), dtype=sl.dtype)
    out[:, :, :, :, :SH] = v.transpose(1, 0, 2, 3, 4)
    return np.ascontiguousarray(out).reshape(NG, 128, CHKG * 2 * FPAD).view(np.float32)


def kernel(x, adj_mat, W_in, b_in, Wl1, Wa1, Wn1, W2, b2, Wl2, Wa2, Wn2,
           Wm1, bm1, Wm2, bm2, Wm3, bm3, _trace=False, _trace_kwargs=None):
    import ml_dtypes

    x = np.asarray(x, dtype=np.float32)
    adj_mat = np.asarray(adj_mat, dtype=np.float32)

    wn1T8 = (np.asarray(Wn1, np.float32).T * WN_SCALE).astype(ml_dtypes.float8_e4m3)
    wn2T8 = (np.asarray(Wn2, np.float32).T * WN_SCALE).astype(ml_dtypes.float8_e4m3)

    adjt = np.ascontiguousarray(adj_mat.transpose(1, 2, 0))  # [i, j, b]
    common = {
        "adjt": adjt,
        "w_int": np.ascontiguousarray(np.vstack([np.asarray(W_in, np.float32).T,
                                                 np.asarray(b_in, np.float32).reshape(1, DH)])),
        "wl1t": np.ascontiguousarray(np.asarray(Wl1, np.float32).T),
        "wa1": np.asarray(Wa1, np.float32).reshape(1, DH).T.copy(),
        "w2t": np.ascontiguousarray(np.asarray(W2, np.float32).T),
        "b2": np.ascontiguousarray(np.asarray(b2, np.float32).reshape(2, DH).T),
        "wl2t": np.ascontiguousarray(np.asarray(Wl2, np.float32).T),
        "wa2": np.asarray(Wa2, np.float32).reshape(1, DH).T.copy(),
        "wm1t": np.ascontiguousarray(np.asarray(Wm1, np.float32).T),
        "bm1": np.ascontiguousarray(np.asarray(bm1, np.float32).reshape(2, DH).T),
        "wm2t": np.ascontiguousarray(np.asarray(Wm2, np.float32).T),
        "bm2": np.asarray(bm2, np.float32).reshape(DH, 1),
        "wm3t": np.ascontiguousarray(np.asarray(Wm3, np.float32).T),
        "bm3": np.asarray(bm3, np.float32).reshape(2, 1),
        "ident": np.eye(128, dtype=np.float32),
        "eye100": np.eye(N, dtype=np.float32),
    }
    in_maps = []
    for c in range(NCORE):
        m = dict(common)
        m["wn1p"] = _pack_wn(wn1T8, c)
        m["wn2p"] = _pack_wn(wn2T8, c)
        m["adj_own"] = np.ascontiguousarray(adj_mat[c])
        m["xt"] = np.ascontiguousarray(np.vstack([x[c].T, np.ones((1, N), np.float32)]))
        in_maps.append(m)

    nc = _get_nc()
    kw = {}
    if _trace:
        kw["trace"] = True
        if _trace_kwargs:
            kw.update(_trace_kwargs)
    res = run_bass_kernel_spmd(nc, in_maps, core_ids=list(range(NCORE)), **kw)
    out = np.stack([res.results[c]["out"] for c in range(NCORE)], axis=0)
    if _trace:
        kernel._last_results = res
    return out


# revision 22
# speedup vs baseline: 1.1338x; 1.0828x over previous
"""GATv2 (2-layer, N=100, B=8) Trainium2 Bass kernel, 8-core SPMD.

Strategy (v4, dense-128 DoubleRow + single-ring bulk DMA + warm PE):
  * The two [10000,10000] f32 lin_n_node matrices dominate.  edge_att_L =
    tanh(inv @ WnL.T) depends only on adj_mat, so both big matmuls are
    tensor-parallel sharded over the output dim: core c owns 1250 columns
    of WnL.T in fp8e4 (x256 scale).  The contraction dim m = 10000 is
    host-packed DENSELY over all 128 partitions: m = 80*p + 2*c + d with
    p in [0,128), chunk c in [0,40), DoubleRow pair d in {0,1} -- every
    PE column-cycle carries 256 live fp8 values (vs 200 for the naive
    [100 x 100] split).  inv (x 2^-2, fp8) is repartitioned on-chip into
    the same order via a 160KB DRAM bounce, and serves as the stationary
    [128, 2, 16] per chunk.  The x2^6 net scale is undone by the tanh's
    input scale.  An AllToAll (bf16) then hands core c the full [10000]
    row for batch c.
  * Weight streaming is 10 group-DMAs of 1.29MB per layer ([128 parts x
    10112B contiguous per partition]) on the sync HWDGE ring alone --
    near line rate.  The scalar (ACT) ring carries only small latency-
    critical DMAs; gpsimd carries e-bounces and collective triggers.
  * A burst of dummy matmuls at t=0 lifts the PE out of the HAM 1.2GHz
    cold state before streaming starts, and the schedule keeps PE gaps
    well under the 3.4us re-throttle window.
  * Both layers stream back-to-back; a2a_1 flies while layer-2 streams;
    attention-1 / o1 / g2 / e2 overlap inside the layer-2 stream.
"""

import sys

for p in ("/opt/trn_rl_repo", "/opt/pypackages"):
    if p not in sys.path:
        sys.path.insert(0, p)

import numpy as np

import concourse.bass as bass
import concourse.mybir as mybir
import concourse.tile as tile
from concourse import bacc
from concourse.bass_utils import run_bass_kernel_spmd

F32 = mybir.dt.float32
BF16 = mybir.dt.bfloat16
FP8 = mybir.dt.float8e4
AF = mybir.ActivationFunctionType
ALU = mybir.AluOpType
DR = mybir.MatmulPerfMode.DoubleRow

N = 100
N2 = N * N
B = 8
NCORE = 8
SH = N2 // NCORE          # 1250 output columns per core
DH = 128                  # hidden dim
INF_ = 64                 # input features
MPAD = 10240              # m padded to 128 * 80
MP = 80                   # m-values per partition
NCHUNK = 40               # DoubleRow chunks (2 m per partition each)
CHKG = 4                  # chunks per DMA group
NG = NCHUNK // CHKG       # 10 group DMAs per layer
FPAD = 1264               # padded k-stride inside a DoubleRow half (16 | 1264)
GROUP_F32 = CHKG * 2 * FPAD // 4  # 2528 f32 per partition per group
IT_SLICES = [(0, 512), (512, 512), (1024, SH - 1024)]  # psum bank slices of 1250
WN_SCALE = 256.0          # host multiplies Wn by 2^8 before fp8 cast
INV_SCALE = 0.25          # kernel multiplies inv by 2^-2 before fp8 cast
TANH_SCALE = 1.0 / (WN_SCALE * INV_SCALE)  # 2^-6 undo on the psum accumulator
CHUNK_I = 10              # i-rows per e-chunk


def build_nc():
    nc = bacc.Bacc(None, num_devices=NCORE)

    # ---- kernel I/O ----
    wn1p = nc.dram_tensor("wn1p", [NG, 128, GROUP_F32], F32, kind="ExternalInput")
    wn2p = nc.dram_tensor("wn2p", [NG, 128, GROUP_F32], F32, kind="ExternalInput")
    adjt = nc.dram_tensor("adjt", [N, N, B], F32, kind="ExternalInput")   # adj[b,i,j] -> [i,j,b]
    adj_own = nc.dram_tensor("adj_own", [N, N], F32, kind="ExternalInput")  # adj[c]
    xt = nc.dram_tensor("xt", [INF_ + 1, N], F32, kind="ExternalInput")      # [x[c].T; ones]
    w_int = nc.dram_tensor("w_int", [INF_ + 1, DH], F32, kind="ExternalInput")
    wl1t = nc.dram_tensor("wl1t", [DH, DH], F32, kind="ExternalInput")
    wa1 = nc.dram_tensor("wa1", [DH, 1], F32, kind="ExternalInput")
    w2t = nc.dram_tensor("w2t", [2 * DH, 2 * DH], F32, kind="ExternalInput")
    b2 = nc.dram_tensor("b2", [DH, 2], F32, kind="ExternalInput")
    wl2t = nc.dram_tensor("wl2t", [2 * DH, DH], F32, kind="ExternalInput")
    wa2 = nc.dram_tensor("wa2", [DH, 1], F32, kind="ExternalInput")
    wm1t = nc.dram_tensor("wm1t", [3 * DH, 2 * DH], F32, kind="ExternalInput")
    bm1 = nc.dram_tensor("bm1", [DH, 2], F32, kind="ExternalInput")
    wm2t = nc.dram_tensor("wm2t", [2 * DH, DH], F32, kind="ExternalInput")
    bm2 = nc.dram_tensor("bm2", [DH, 1], F32, kind="ExternalInput")
    wm3t = nc.dram_tensor("wm3t", [DH, 2], F32, kind="ExternalInput")
    bm3 = nc.dram_tensor("bm3", [2, 1], F32, kind="ExternalInput")
    ident = nc.dram_tensor("ident", [128, 128], F32, kind="ExternalInput")
    eye100 = nc.dram_tensor("eye100", [N, N], F32, kind="ExternalInput")  # 1.0*I
    out_ext = nc.dram_tensor("out", [N, 2], F32, kind="ExternalOutput")

    with tile.TileContext(nc) as tc:
        with (
            tc.tile_pool(name="const", bufs=1) as cpool,
            tc.tile_pool(name="prep", bufs=1) as prep,
            tc.tile_pool(name="state", bufs=1) as state,
            tc.tile_pool(name="work", bufs=2) as work,
            tc.tile_pool(name="epool", bufs=3) as epool,
            tc.tile_pool(name="wn", bufs=8) as wnp,
            tc.tile_pool(name="psbig", bufs=1, space="PSUM") as psbig,
            tc.tile_pool(name="pssm", bufs=5, space="PSUM") as pssm,
            tc.tile_pool(name="dram", bufs=1, space="DRAM") as dram,
        ):
            # =============================================================
            # Bulk weight streams: all 20 group DMAs on the sync HWDGE ring
            # (FIFO, paced by the wn pool's 8 buffers).  Nothing else ever
            # rides this ring, so descriptor supply is continuous.
            # =============================================================
            _wn_dram = {1: wn1p, 2: wn2p}
            _wn_tiles = {1: {}, 2: {}}

            def dma_group(tag, g):
                wtile = wnp.tile([128, CHKG, 2 * FPAD // 4], F32, name="wn")
                nc.sync.dma_start(
                    wtile[:],
                    _wn_dram[tag][g].rearrange("p (a b) -> p a b", a=CHKG),
                )
                _wn_tiles[tag][g] = wtile

            for g in range(NG):
                dma_group(1, g)
            for g in range(NG):
                dma_group(2, g)

            # ---- critical constants (scalar ring; small + early) ----
            def cload(name, dt_, shape, src, eng=None):
                t = cpool.tile(shape, dt_, name=name)
                (eng or nc.scalar).dma_start(t[:], src[:])
                return t

            def cload_kt(name, src, kt, m, eng=None):
                t = cpool.tile([128, kt, m], F32, name=name)
                (eng or nc.gpsimd).dma_start(t[:], src[:].rearrange("(k p) m -> p k m", p=128))
                return t

            adj_sb = cload("adj_sb", F32, [N, N, B], adjt)
            eye_sb = cload("eye_sb", F32, [N, N], eye100)
            adjo_sb = cload("adjo_sb", F32, [N, N], adj_own)
            xt_sb = cload("xt_sb", F32, [INF_ + 1, N], xt)
            w_int_sb = cload("w_int_sb", F32, [INF_ + 1, DH], w_int)
            wl1t_sb = cload("wl1t_sb", F32, [DH, DH], wl1t)
            wa1_sb = cload("wa1_sb", F32, [DH, 1], wa1)
            id_sb = cload("id_sb", F32, [128, 128], ident)

            # ---- PE warmup: lift HAM out of the 1.2GHz cold state ----
            warm = cpool.tile([128, 512], F32, name="warm")
            nc.vector.memset(warm[:], 0.0)
            for _ in range(8):
                pw = pssm.tile([16, 512], F32, name="ps")
                nc.tensor.matmul(pw[:], warm[:, 0:16], warm[:], start=True, stop=True)

            # preload ACT LUTs (sqrt/tanh/exp) so first real use isn't blocked
            tscr = work.tile([1, 4], F32, name="tscr")
            nc.vector.memset(tscr[:], 1.0)
            nc.scalar.sqrt(tscr[:], tscr[:])
            nc.scalar.activation(tscr[:], tscr[:], AF.Tanh)
            nc.scalar.activation(tscr[:], tscr[:], AF.Exp)

            # =============================================================
            # Stage A: adj -> inv8 (adj entries are {0,1}).
            #   inv = 0.25 * norm_row * (adj + I.(2 - 7/3 adj))
            #   norm_row^2 = rowsum(adj) + diag(adj) + 0.25
            # Then repartition [100,100,16] -> [128, 80, 16] (m = 80p + q)
            # through a 160KB DRAM bounce so every partition carries live m.
            # =============================================================
            eyeb = eye_sb[:, :, None].to_broadcast([N, N, B])
            s_t = prep.tile([N, N, B], F32, name="s_t")
            nc.vector.tensor_scalar(s_t[:], adj_sb[:], -7.0 / 3.0, 2.0, ALU.mult, ALU.add)
            nc.vector.tensor_tensor(s_t[:], eyeb, s_t[:], ALU.mult)
            w_t = s_t
            nc.vector.tensor_tensor(w_t[:], adj_sb[:], s_t[:], ALU.add)

            d1 = prep.tile([N, N, B], F32, name="d1")
            nc.vector.tensor_tensor(d1[:], eyeb, adj_sb[:], ALU.mult)
            nc.vector.tensor_tensor(d1[:], adj_sb[:], d1[:], ALU.add)
            nsq = prep.tile([N, B], F32, name="nsq")
            nc.vector.tensor_reduce(nsq[:], d1[:].rearrange("i j b -> i b j"),
                                    axis=mybir.AxisListType.X, op=ALU.add)
            nc.vector.tensor_scalar(nsq[:], nsq[:], 1.0, 0.25, ALU.mult, ALU.add)
            # nwt = 0.25*sqrt(nsq) via the ACT input scale: sqrt(nsq/16)
            nwt = prep.tile([N, B], F32, name="nwt")
            nc.scalar.activation(nwt[:], nsq[:], AF.Sqrt,
                                 scale=INV_SCALE * INV_SCALE)

            inv8 = state.tile([N, N, 16], FP8, name="inv8")
            nc.vector.memset(inv8[:], 0.0)
            nc.vector.tensor_tensor(inv8[:, :, 0:B], w_t[:],
                                    nwt[:, None, :].to_broadcast([N, N, B]), ALU.mult)

            # bounce: SBUF [100,100,16] -> DRAM flat (m,b) -> SBUF [128,80,16]
            xdram = dram.tile([N2, 16], FP8)
            nc.scalar.dma_start(
                xdram[:].rearrange("(i j) b -> i j b", j=N), inv8[:]
            )
            inv8r = state.tile([128, MP, 16], FP8, name="inv8r")
            nc.vector.memset(inv8r[:], 0.0)
            nc.scalar.dma_start(
                inv8r[0:125, :, :],
                xdram[:].rearrange("(p q) b -> p q b", q=MP),
            )

            # own-batch mask + additive -10000 bias for the e-side
            mb0 = prep.tile([N, N], F32, name="mb0")
            nc.vector.tensor_tensor(mb0[:], adjo_sb[:], eye_sb[:], ALU.add)
            maskb = prep.tile([N, N], F32, name="maskb")
            nc.vector.tensor_scalar(maskb[:], mb0[:], 1.0, None, ALU.min)
            eqz = prep.tile([N, N], F32, name="eqz")
            nc.vector.tensor_scalar(eqz[:], mb0[:], 0.0, None, ALU.is_equal)
            negm = prep.tile([N, N], F32, name="negm")
            nc.vector.tensor_scalar(negm[:], eqz[:], -10000.0, None, ALU.mult)

            # =============================================================
            # Batch-side prologue: h_inT, g1T, gnm1
            # =============================================================
            ps = pssm.tile([DH, N], F32, name="ps")
            nc.tensor.matmul(ps[:], w_int_sb[:], xt_sb[:], start=True, stop=True)
            h_inT = state.tile([DH, N], F32, name="h_inT")
            nc.vector.tensor_copy(h_inT[:], ps[:])

            ps = pssm.tile([DH, N], F32, name="ps")
            nc.tensor.matmul(ps[:], wl1t_sb[:], h_inT[:], start=True, stop=True)
            g1T = state.tile([DH, N], F32, name="g1T")
            nc.vector.tensor_copy(g1T[:], ps[:])

            def g_node_major(gT, tag):
                psg = pssm.tile([N, DH], F32, name="ps")
                nc.tensor.transpose(psg[:], gT[:], id_sb[:, :])
                gnm = state.tile([N, DH], F32, name=f"gnm{tag}")
                nc.vector.tensor_copy(gnm[:], psg[:])
                return gnm

            gnm1 = g_node_major(g1T, 1)
            g1Tb = state.tile([DH, N], BF16, name="g1Tb")
            nc.vector.tensor_copy(g1Tb[:], g1T[:])
            wa1b = cpool.tile([DH, 1], BF16, name="wa1b")
            nc.vector.tensor_copy(wa1b[:], wa1_sb[:])

            # =============================================================
            # e[i,j] = Wa . tanh(g_i + g_j), bounced to DRAM for the
            # [1, i*j] -> [i, j] repartition.  The add/tanh pipe runs in
            # bf16 (2x DVE rate); adds alternate gpsimd/DVE and the psum
            # evacuations alternate DVE/ACT so no single engine gates the
            # PE's interleaved e-matmuls.
            # =============================================================
            def e_front(gTb, ci, tmp2_name="etmp2", tmp2_bufs=None):
                i0 = ci * CHUNK_I
                tmp = epool.tile([DH, CHUNK_I, N], BF16, name="etmp")
                nc.vector.tensor_tensor(
                    tmp[:],
                    gTb[:, i0 : i0 + CHUNK_I, None].to_broadcast([DH, CHUNK_I, N]),
                    gTb[:, None, :].to_broadcast([DH, CHUNK_I, N]),
                    ALU.add,
                )
                tmp2 = epool.tile([DH, CHUNK_I, N], BF16, name=tmp2_name,
                                  bufs=tmp2_bufs)
                nc.scalar.activation(tmp2[:], tmp[:], AF.Tanh)
                return tmp2

            def e_back(tmp2, wa_b, e_ij, ci):
                i0 = ci * CHUNK_I
                half = CHUNK_I // 2
                eb = epool.tile([1, CHUNK_I * N], F32, name="ebounce", bufs=2)
                for h in range(2):
                    pe = pssm.tile([1, half * N], F32, name="ps")
                    nc.tensor.matmul(
                        pe[:], wa_b[:],
                        tmp2[:, h * half : (h + 1) * half, :].rearrange("p a b -> p (a b)"),
                        start=True, stop=True,
                    )
                    # psum evacuation: gpsimd has no PSUM route, so DVE takes
                    # one half and ACT the other.  Backs are emitted only at
                    # points where a PE dependency on those queues is
                    # harmless (their upstream adds/tanhs are long done).
                    dst = eb[:, h * half * N : (h + 1) * half * N]
                    if h == 0:
                        nc.vector.tensor_copy(dst, pe[:])
                    else:
                        nc.scalar.activation(dst, pe[:], AF.Identity)
                # SBUF->SBUF repartition: [1, i*j] on one partition -> [i, j]
                nc.gpsimd.dma_start(e_ij[i0 : i0 + CHUNK_I, :], eb[0:1, :])

            # =============================================================
            # TP streams: fp8 Wn chunks, dense-128 DoubleRow accumulation
            # =============================================================
            _accs = {}

            def wn_accs():
                if "s" not in _accs:
                    _accs["s"] = [
                        psbig.tile([16, 512], F32, name=f"acc_{it}")
                        for it in range(3)
                    ]
                return _accs["s"]

            def mm_group(tag, g):
                wtile = _wn_tiles[tag][g]
                accs = wn_accs()
                for q in range(CHKG):
                    c = g * CHKG + q
                    first = c == 0
                    last = c == NCHUNK - 1
                    rhs_pair = wtile[:, q, :].bitcast(FP8).rearrange(
                        "p (two f) -> p two f", two=2
                    )
                    for it, (o, w) in enumerate(IT_SLICES):
                        nc.tensor.matmul(
                            accs[it][:, :w],
                            inv8r[:, 2 * c : 2 * c + 2, :],
                            rhs_pair[:, :, o : o + w],
                            start=first,
                            stop=last,
                            perf_mode=DR,
                        )

            def a2a_start(tag):
                accs = wn_accs()
                ea = state.tile([B, SH], BF16, name=f"ea{tag}")
                for it, (o, w) in enumerate(IT_SLICES):
                    nc.scalar.activation(ea[:, o : o + w], accs[it][0:B, :w],
                                         AF.Tanh, scale=TANH_SCALE)
                cc_in = dram.tile([B, SH], BF16)
                cc_out = dram.tile([B, SH], BF16)
                nc.scalar.dma_start(cc_in[:], ea[:])
                nc.gpsimd.collective_compute(
                    "AllToAll",
                    ALU.bypass,
                    replica_groups=[list(range(NCORE))],
                    ins=[cc_in[:].opt()],
                    outs=[cc_out[:].opt()],
                )
                return cc_out

            def a2a_finish(tag, cc_out):
                ea_ij = state.tile([N, N], BF16, name=f"eaij{tag}")
                nc.scalar.dma_start(
                    ea_ij[:], cc_out[:].rearrange("b f -> (b f)").rearrange("(i j) -> i j", j=N)
                )
                return ea_ij

            # =============================================================
            # attention + aggregation (batch side)
            # =============================================================
            def attn_dve(em, ea_ij, tag):
                ef = work.tile([N, N], F32, name=f"ef{tag}")
                nc.vector.tensor_mul(out=ef[:], in0=em[:], in1=ea_ij[:])
                nc.vector.tensor_add(out=ef[:], in0=ef[:], in1=negm[:])
                # row softmax (no max-subtraction: |ef| <= ~4 or exactly -1e4)
                aw = work.tile([N, N], F32, name=f"aw{tag}")
                nc.scalar.activation(aw[:], ef[:], AF.Exp)
                ssum = work.tile([N, 1], F32, name=f"ssum{tag}")
                nc.vector.tensor_reduce(ssum[:], aw[:], axis=mybir.AxisListType.X, op=ALU.add)
                rsum = work.tile([N, 1], F32, name=f"rsum{tag}")
                nc.vector.reciprocal(rsum[:], ssum[:])
                nc.vector.tensor_scalar_mul(aw[:], aw[:], rsum[:, 0:1])
                return aw

            def attn_pe(aw, gnm, tag):
                pst = pssm.tile([N, N], F32, name="ps")
                nc.tensor.transpose(pst[:], aw[:], id_sb[:N, :N])
                awT = work.tile([N, N], F32, name=f"awT{tag}")
                nc.vector.tensor_copy(awT[:], pst[:])
                psr = pssm.tile([DH, N], F32, name="ps")
                nc.tensor.matmul(psr[:], gnm[:], awT[:], start=True, stop=True)
                return psr

            # ---------------- schedule ----------------
            # Layer-1 stream with e1 chunks interleaved (their chain runs on
            # gpsimd/DVE/ACT which are otherwise idle during the DMA-paced
            # stream).
            # Layer-1 stream: only the e1 fronts (DVE add + ACT tanh) run
            # here; the PE-coupled backs wait for layer 2's stream bubbles.
            e1_ij = state.tile([N, N], F32, name="eij1")
            e1tmp2 = []
            for g in range(NG):
                mm_group(1, g)
                e1tmp2.append(e_front(g1Tb, g, tmp2_name="e1tmp2",
                                      tmp2_bufs=NG))

            cc1_out = a2a_start(1)

            # deferred constants (gpsimd is clear of e1 adds by now)
            w2t_sb = cload_kt("w2t_sb", w2t, 2, 2 * DH)      # [128, 2, 256]
            b2_sb = cload("b2_sb", F32, [DH, 2], b2, eng=nc.gpsimd)
            wl2t_sb = cload_kt("wl2t_sb", wl2t, 2, DH)       # [128, 2, 128]
            wa2_sb = cload("wa2_sb", F32, [DH, 1], wa2, eng=nc.gpsimd)
            wm1t_sb = cload_kt("wm1t_sb", wm1t, 3, 2 * DH)   # [128, 3, 256]
            bm1_sb = cload("bm1_sb", F32, [DH, 2], bm1, eng=nc.gpsimd)
            wm2t_sb = cload_kt("wm2t_sb", wm2t, 2, DH)       # [128, 2, 128]
            bm2_sb = cload("bm2_sb", F32, [DH, 1], bm2, eng=nc.gpsimd)
            wm3t_sb = cload("wm3t_sb", F32, [DH, 2], wm3t, eng=nc.gpsimd)
            bm3_sb = cload("bm3_sb", F32, [2, 1], bm3, eng=nc.gpsimd)

            # Layer-2 stream back-to-back (a2a_1 flies underneath).  The e1
            # backs ride the PE's DMA-pacing bubbles in groups 0-4; the
            # attention-1 chain slots in after group 7: a2a_1 has landed by
            # then, so the PE takes no stall and the e2 chain starts while
            # the last groups still stream.
            for g in range(NG):
                mm_group(2, g)
                if g < 5:
                    e_back(e1tmp2[2 * g], wa1b, e1_ij, 2 * g)
                    e_back(e1tmp2[2 * g + 1], wa1b, e1_ij, 2 * g + 1)
                if g == 5:
                    em1 = state.tile([N, N], F32, name="em1")
                    nc.vector.tensor_mul(out=em1[:], in0=e1_ij[:], in1=maskb[:])
                    ea1_ij = a2a_finish(1, cc1_out)
                    aw1 = attn_dve(em1, ea1_ij, 1)
                if g == 7:
                    psr1 = attn_pe(aw1, gnm1, 1)
                    out1T = state.tile([DH, N], F32, name="out1T")
                    nc.scalar.activation(out1T[:], psr1[:], AF.Tanh)

                    # o1T = tanh(W2 @ [out1; h_in] + b2), M split in 2 halves
                    o1T = []
                    for mh in range(2):
                        pso = pssm.tile([DH, N], F32, name="ps")
                        mslc = slice(mh * DH, (mh + 1) * DH)
                        nc.tensor.matmul(pso[:], w2t_sb[:, 0, mslc], out1T[:], start=True, stop=False)
                        nc.tensor.matmul(pso[:], w2t_sb[:, 1, mslc], h_inT[:], start=False, stop=True)
                        t = state.tile([DH, N], F32, name=f"o1T_{mh}")
                        nc.scalar.activation(t[:], pso[:], AF.Tanh, bias=b2_sb[:, mh : mh + 1])
                        o1T.append(t)

                    # g2T = Wl2 @ o1T  (K = 256)
                    psg2 = pssm.tile([DH, N], F32, name="ps")
                    nc.tensor.matmul(psg2[:], wl2t_sb[:, 0, :], o1T[0][:], start=True, stop=False)
                    nc.tensor.matmul(psg2[:], wl2t_sb[:, 1, :], o1T[1][:], start=False, stop=True)
                    g2T = state.tile([DH, N], F32, name="g2T")
                    nc.vector.tensor_copy(g2T[:], psg2[:])
                    gnm2 = g_node_major(g2T, 2)
                    g2Tb = state.tile([DH, N], BF16, name="g2Tb")
                    nc.vector.tensor_copy(g2Tb[:], psg2[:])
                    wa2b = cpool.tile([DH, 1], BF16, name="wa2b")
                    nc.vector.tensor_copy(wa2b[:], wa2_sb[:])

            # a2a_2 kicks off the moment the last accumulation lands; its
            # ACT/scalar ops are queued ahead of the e2 tanh chain so the e2
            # pipeline can never delay the collective.
            cc2_out = a2a_start(2)

            # e2 chunks fill the a2a_2 flight window.  All add/tanh fronts
            # are emitted before any psum evacuation so the DVE never
            # inherits a PE dependency mid-pipeline; the h0 evacuations then
            # ride the DVE (idle in the tail) so gpsimd carries only half.
            e2_ij = state.tile([N, N], F32, name="eij2")
            NEC = N // CHUNK_I
            tmp2s = [e_front(g2Tb, ci, tmp2_name="e2tmp2", tmp2_bufs=NEC)
                     for ci in range(NEC)]
            for ci in range(NEC):
                e_back(tmp2s[ci], wa2b, e2_ij, ci)

            em2 = state.tile([N, N], F32, name="em2")
            nc.vector.tensor_mul(out=em2[:], in0=e2_ij[:], in1=maskb[:])

            ea2_ij = a2a_finish(2, cc2_out)

            # tiny warm-keeper matmuls so the HAM clock gate stays open
            # across the a2a_2 landing gap
            for _ in range(6):
                pw = pssm.tile([16, 512], F32, name="ps")
                nc.tensor.matmul(pw[:, 0:64], warm[:, 0:16], warm[:, 0:64],
                                 start=True, stop=True)

            aw2 = attn_dve(em2, ea2_ij, 2)
            psr2 = attn_pe(aw2, gnm2, 2)
            out2T = state.tile([DH, N], F32, name="out2T")
            nc.scalar.activation(out2T[:], psr2[:], AF.Tanh)

            # MLP: q1 = relu(Wm1 @ [out2; o1] + bm1)  (K=384, M=256)
            o2T_parts = [out2T, o1T[0], o1T[1]]
            q1T = []
            for mh in range(2):
                psq = pssm.tile([DH, N], F32, name="ps")
                mslc = slice(mh * DH, (mh + 1) * DH)
                for kt in range(3):
                    nc.tensor.matmul(
                        psq[:], wm1t_sb[:, kt, mslc], o2T_parts[kt][:],
                        start=(kt == 0), stop=(kt == 2),
                    )
                t = state.tile([DH, N], F32, name=f"q1T_{mh}")
                nc.scalar.activation(t[:], psq[:], AF.Relu, bias=bm1_sb[:, mh : mh + 1])
                q1T.append(t)

            # q2 = relu(Wm2 @ q1 + bm2)  (K=256, M=128)
            psq2 = pssm.tile([DH, N], F32, name="ps")
            nc.tensor.matmul(psq2[:], wm2t_sb[:, 0, :], q1T[0][:], start=True, stop=False)
            nc.tensor.matmul(psq2[:], wm2t_sb[:, 1, :], q1T[1][:], start=False, stop=True)
            q2T = state.tile([DH, N], F32, name="q2T")
            nc.scalar.activation(q2T[:], psq2[:], AF.Relu, bias=bm2_sb[:, 0:1])

            # q3 = Wm3 @ q2 + bm3  [2, 100]
            psq3 = pssm.tile([2, N], F32, name="ps")
            nc.tensor.matmul(psq3[:], wm3t_sb[:], q2T[:], start=True, stop=True)
            q3T = state.tile([2, N], F32, name="q3T")
            nc.scalar.activation(q3T[:], psq3[:], AF.Identity, bias=bm3_sb[:, 0:1])

            # transpose -> [100, 2], softmax over classes (free dim)
            psf = pssm.tile([N, 2], F32, name="ps")
            nc.tensor.transpose(psf[:], q3T[:], id_sb[:2, :2])
            qf = work.tile([N, 2], F32, name="qf")
            nc.vector.tensor_copy(qf[:], psf[:])
            fm = work.tile([N, 1], F32, name="fm")
            nc.vector.tensor_reduce(fm[:], qf[:], axis=mybir.AxisListType.X,
                                    op=ALU.max, negate=True)
            pf = work.tile([N, 2], F32, name="pf")
            nc.scalar.activation(pf[:], qf[:], AF.Exp, bias=fm[:, 0:1])
            sf = work.tile([N, 1], F32, name="sf")
            nc.vector.tensor_reduce(sf[:], pf[:], axis=mybir.AxisListType.X, op=ALU.add)
            rf = work.tile([N, 1], F32, name="rf")
            nc.vector.reciprocal(rf[:], sf[:])
            outp = work.tile([N, 2], F32, name="outp")
            nc.vector.tensor_scalar_mul(outp[:], pf[:], rf[:, 0:1])
            nc.scalar.dma_start(out_ext[:], outp[:])

    nc.compile()
    return nc


_NC_CACHE = None


def _get_nc():
    global _NC_CACHE
    if _NC_CACHE is None:
        _NC_CACHE = build_nc()
    return _NC_CACHE


def _pack_wn(WnT8, c):
    """[N2, N2] fp8 WnT -> core c's [NG, 128, GROUP_F32] dense-128 pack.

    m = 80*p + 2*(g*CHKG + q) + d; tile byte layout per partition is
    [q(4), d(2), k(1264 fp8, 1250 valid)].
    """
    sl = WnT8[:, c * SH : (c + 1) * SH]  # [10000, 1250]
    arr = np.zeros((MPAD, SH), dtype=sl.dtype)
    arr[:N2] = sl
    v = arr.reshape(128, NCHUNK, 2, SH)            # p, c, d, k
    v = v.reshape(128, NG, CHKG, 2, SH)            # p, g, q, d, k
    out = np.zeros((NG, 128, CHKG, 2, FPAD), dtype=sl.dtype)
    out[:, :, :, :, :SH] = v.transpose(1, 0, 2, 3, 4)
    return np.ascontiguousarray(out).reshape(NG, 128, CHKG * 2 * FPAD).view(np.float32)


def kernel(x, adj_mat, W_in, b_in, Wl1, Wa1, Wn1, W2, b2, Wl2, Wa2, Wn2,
           Wm1, bm1, Wm2, bm2, Wm3, bm3, _trace=False, _trace_kwargs=None):
    import ml_dtypes

    x = np.asarray(x, dtype=np.float32)
    adj_mat = np.asarray(adj_mat, dtype=np.float32)

    wn1T8 = (np.asarray(Wn1, np.float32).T * WN_SCALE).astype(ml_dtypes.float8_e4m3)
    wn2T8 = (np.asarray(Wn2, np.float32).T * WN_SCALE).astype(ml_dtypes.float8_e4m3)

    adjt = np.ascontiguousarray(adj_mat.transpose(1, 2, 0))  # [i, j, b]
    common = {
        "adjt": adjt,
        "w_int": np.ascontiguousarray(np.vstack([np.asarray(W_in, np.float32).T,
                                                 np.asarray(b_in, np.float32).reshape(1, DH)])),
        "wl1t": np.ascontiguousarray(np.asarray(Wl1, np.float32).T),
        "wa1": np.asarray(Wa1, np.float32).reshape(1, DH).T.copy(),
        "w2t": np.ascontiguousarray(np.asarray(W2, np.float32).T),
        "b2": np.ascontiguousarray(np.asarray(b2, np.float32).reshape(2, DH).T),
        "wl2t": np.ascontiguousarray(np.asarray(Wl2, np.float32).T),
        "wa2": np.asarray(Wa2, np.float32).reshape(1, DH).T.copy(),
        "wm1t": np.ascontiguousarray(np.asarray(Wm1, np.float32).T),
        "bm1": np.ascontiguousarray(np.asarray(bm1, np.float32).reshape(2, DH).T),
        "wm2t": np.ascontiguousarray(np.asarray(Wm2, np.float32).T),
        "bm2": np.asarray(bm2, np.float32).reshape(DH, 1),
        "wm3t": np.ascontiguousarray(np.asarray(Wm3, np.float32).T),
        "bm3": np.asarray(bm3, np.float32).reshape(2, 1),
        "ident": np.eye(128, dtype=np.float32),
        "eye100": np.eye(N, dtype=np.float32),
    }
    in_maps = []
    for c in range(NCORE):
        m = dict(common)
        m["wn1p"] = _pack_wn(wn1T8, c)
        m["wn2p"] = _pack_wn(wn2T8, c)
        m["adj_own"] = np.ascontiguousarray(adj_mat[c])
        m["xt"] = np.ascontiguousarray(np.vstack([x[c].T, np.ones((1, N), np.float32)]))
        in_maps.append(m)

    nc = _get_nc()
    kw = {}
    if _trace:
        kw["trace"] = True
        if _trace_kwargs:
            kw.update(_trace_kwargs)
    res = run_bass_kernel_spmd(nc, in_maps, core_ids=list(range(NCORE)), **kw)
    out = np.stack([res.results[c]["out"] for c in range(NCORE)], axis=0)
    if _trace:
        kernel._last_results = res
    return out
